# revision 16
# baseline (speedup 1.0000x reference)
"""Trainium2 Bass kernel for loss = sum((X[:,None]*A - I)**2), N=8192.

Algebraic decomposition (avoids materializing the residual):
    loss = sum_i x_i^2 * r_i  -  2*sum_i x_i*a_ii  +  N
where r_i = sum_j a_ij^2 (row sums of squares of A).

Device work is pure streaming: each core reads its 32 MiB row-shard of A
once and produces per-chunk row sums of squares. The x^2 weighting, the
diagonal term, and all cross-partition/core reduction run on the host in
float64 from the tiny [128, n_chunks] result.

Perf structure (evolved via perfetto traces; v1 single-ring was 130 us):
  - Each of the 8 cores has its HBM stack pair to itself (trn2.8x1),
    so two concurrent DMA queues sustain ~430 GB/s aggregate (fabric
    limit ~435) vs ~340 GB/s for one. Ring A = nc.sync (qSPDynamicHW)
    carries row-tiles 0,2,4,6; ring B = nc.scalar (qActDynamicHW)
    carries 1,3,5,7.
  - Compute is split per-ring so neither engine waits on the other
    ring's (unfairly scheduled, oscillating) completions: ScalarE
    activation(Square, accum_out) consumes ring A; VectorE
    scalar_tensor_tensor(a*1.0*a, accum_out) consumes ring B in a
    single pass (measured DVE fp32 is ~118 G elem/s per pass; ACT is
    ~110 G elem/s; each ring delivers only ~54 G elem/s).
    NOTE: tensor_tensor_reduce is NOT used - its accumulator read
    lowers to a raw InstISA op that crashes the HW path.
  - 1 MiB chunks with 8 buffers per ring give each DMA queue 8 MiB of
    issue leash, so buffer-reuse gating (issue k+8 waits compute k)
    never starves a queue even through SDMA round-robin oscillations.
  - Per-ring taper [1024,512,512] so the final compute after the last
    byte lands is <1 us. Separate accumulator tiles per engine (a
    shared tile would make Tile serialize the engines against each
    other).
"""

import numpy as np

import concourse.bacc as bacc
import concourse.mybir as mybir
from concourse.tile import TileContext
from concourse.bass_utils import run_bass_kernel_spmd

N = 8192
NCORES = 8
ROWS = N // NCORES  # 1024 rows per core
P = 128  # SBUF partitions
TILES = ROWS // P  # 8 row-tiles of 128 rows per core

_DT = mybir.dt.float32

BIG = 2048  # big-chunk width (1 MiB)
TAPER = [1024, 512, 512]
assert sum(TAPER) == BIG
NBUF = 8  # big-pool buffers per ring


def _ring_chunks(tiles):
    ch = []
    for i, t in enumerate(tiles):
        last_tile = i == len(tiles) - 1
        ncols = N - BIG if last_tile else N
        for off in range(0, ncols, BIG):
            ch.append((t, off, BIG))
    off = N - BIG
    for w in TAPER:
        ch.append((tiles[-1], off, w))
        off += w
    return ch

CHUNKS_A = _ring_chunks([0, 2, 4, 6])
CHUNKS_B = _ring_chunks([1, 3, 5, 7])
NCHA = len(CHUNKS_A)  # 18
NCH = NCHA + len(CHUNKS_B)  # 36


def build_nc():
    nc = bacc.Bacc("TRN2", target_bir_lowering=False)

    a_shard = nc.dram_tensor("a_shard", [ROWS, N], _DT, kind="ExternalInput")
    out = nc.dram_tensor("out", [P, NCH], _DT, kind="ExternalOutput")

    a_tiles = a_shard.rearrange("(t p) n -> t p n", p=P)

    with TileContext(nc) as tc:
        with (
            tc.tile_pool(name="bigA", bufs=NBUF) as bigA,
            tc.tile_pool(name="bigB", bufs=NBUF) as bigB,
            tc.tile_pool(name="t1024", bufs=2) as t1024,
            tc.tile_pool(name="t512", bufs=4) as t512,
            tc.tile_pool(name="small", bufs=1) as small,
        ):
            tpools = {1024: t1024, 512: t512}
            racc_a = small.tile([P, NCHA], _DT, tag="racc_a")
            racc_v = small.tile([P, NCH - NCHA], _DT, tag="racc_v")
            dummy = small.tile([P, 1], _DT, tag="dummy")
            vout = small.tile([P, BIG], _DT, tag="vout")

            def alloc(ring_pool, ring_tag, w):
                if w == BIG:
                    return ring_pool.tile(
                        [P, w], _DT, tag=ring_tag, name=f"at_{ring_tag}"
                    )
                return tpools[w].tile(
                    [P, w], _DT, tag=f"{ring_tag}{w}", name=f"at_{ring_tag}{w}"
                )

            atA = []
            for t, c0, w in CHUNKS_A:
                at = alloc(bigA, "a", w)
                nc.sync.dma_start(out=at[:], in_=a_tiles[t][:, c0 : c0 + w])
                atA.append(at)

            atB = []

            def issue_b(k):
                t, c0, w = CHUNKS_B[k]
                at = alloc(bigB, "b", w)
                nc.scalar.dma_start(out=at[:], in_=a_tiles[t][:, c0 : c0 + w])
                atB.append(at)

            def act(tile_ap, col):
                nc.scalar.activation(
                    out=dummy.broadcast_to(tile_ap.shape),
                    in_=tile_ap[:],
                    func=mybir.ActivationFunctionType.Square,
                    accum_out=racc_a[:, col : col + 1],
                )

            def dve(k):
                at = atB[k]
                w = at.shape[1]
                nc.vector.scalar_tensor_tensor(
                    out=vout[:, :w],
                    in0=at[:],
                    scalar=1.0,
                    in1=at[:],
                    op0=mybir.AluOpType.mult,
                    op1=mybir.AluOpType.mult,
                    accum_out=racc_v[:, k : k + 1],
                )

            for k in range(NBUF):
                issue_b(k)
            # ScalarE consumes ring A only; VectorE consumes ring B only.
            # Ring-B DMA triggers ride the ACT stream, each emitted ahead
            # of the activation that would otherwise delay it.
            for k in range(NCHA):
                if k + NBUF < NCHA:
                    issue_b(k + NBUF)
                act(atA[k], k)
                dve(k)

            nc.sync.dma_start(out=out[:, :NCHA], in_=racc_a[:])
            nc.scalar.dma_start(out=out[:, NCHA:], in_=racc_v[:])

    nc.compile()
    return nc


_nc_cache = {}


def _get_nc():
    if "nc" not in _nc_cache:
        _nc_cache["nc"] = build_nc()
    return _nc_cache["nc"]


# racc column -> row-tile index, for the host fold.
_COL_TILE = np.array([t for t, _, _ in CHUNKS_A] + [t for t, _, _ in CHUNKS_B])


def _run(inputs, trace=False):
    X = np.ascontiguousarray(np.asarray(inputs["X"], dtype=np.float32))
    A = np.ascontiguousarray(np.asarray(inputs["A"], dtype=np.float32))

    nc = _get_nc()
    in_maps = [
        {"a_shard": A[c * ROWS : (c + 1) * ROWS]} for c in range(NCORES)
    ]
    res = run_bass_kernel_spmd(
        nc, in_maps, core_ids=list(range(NCORES)), trace=trace
    )

    # Host epilogue in float64: fold chunk partials per tile, weight by
    # x^2, add the diagonal term.
    X64 = X.astype(np.float64)
    total = 0.0
    for c in range(NCORES):
        r = res.results[c]["out"].astype(np.float64)  # [P, NCH]
        rt = np.zeros((P, TILES), dtype=np.float64)
        for col in range(NCH):
            rt[:, _COL_TILE[col]] += r[:, col]
        # x for row-tile t, partition p is X[core*ROWS + t*128 + p]
        xc = X64[c * ROWS : (c + 1) * ROWS].reshape(TILES, P).T  # [P, T]
        total += (xc * xc * rt).sum()

    d64 = np.asarray(A.diagonal(), dtype=np.float64)
    total += -2.0 * float(X64 @ d64) + float(N)
    return np.float32(total), res


def kernel(**inputs):
    out, _ = _run(inputs, trace=False)
    return out


# revision 17
# speedup vs baseline: 1.1910x; 1.1910x over previous
"""v4-noDVE bisect: dual-ring 2MiB chunks, upfront issue, all-ACT compute."""

import numpy as np

import concourse.bacc as bacc
import concourse.mybir as mybir
from concourse.tile import TileContext
from concourse.bass_utils import run_bass_kernel_spmd

N = 8192
NCORES = 8
ROWS = N // NCORES
P = 128
TILES = ROWS // P

_DT = mybir.dt.float32

BIG = 4096
TAPER = [2048, 1024, 512, 512]
assert sum(TAPER) == BIG


def _ring_chunks(tiles):
    ch = []
    for t in tiles[:-1]:
        ch.append((t, 0, BIG))
        ch.append((t, BIG, BIG))
    t = tiles[-1]
    ch.append((t, 0, BIG))
    off = BIG
    for w in TAPER:
        ch.append((t, off, w))
        off += w
    return ch

CHUNKS_A = _ring_chunks([0, 2, 4, 6])
CHUNKS_B = _ring_chunks([1, 3, 5, 7])
NCHA = len(CHUNKS_A)  # 11
NCH = NCHA + len(CHUNKS_B)  # 22


def build_nc():
    nc = bacc.Bacc("TRN2", target_bir_lowering=False)

    a_shard = nc.dram_tensor("a_shard", [ROWS, N], _DT, kind="ExternalInput")
    out = nc.dram_tensor("out", [P, NCH], _DT, kind="ExternalOutput")

    a_tiles = a_shard.rearrange("(t p) n -> t p n", p=P)

    with TileContext(nc) as tc:
        with (
            tc.tile_pool(name="bigA", bufs=3) as bigA,
            tc.tile_pool(name="bigB", bufs=4) as bigB,
            tc.tile_pool(name="t2048", bufs=2) as t2048,
            tc.tile_pool(name="t1024", bufs=2) as t1024,
            tc.tile_pool(name="t512", bufs=4) as t512,
            tc.tile_pool(name="small", bufs=1) as small,
        ):
            tpools = {2048: t2048, 1024: t1024, 512: t512}
            # Separate accumulators per engine so Tile never serializes
            # ScalarE against VectorE through a shared tile.
            racc_a = small.tile([P, NCHA], _DT, tag="racc_a")
            racc_v = small.tile([P, NCH - NCHA], _DT, tag="racc_v")
            dummy = small.tile([P, 1], _DT, tag="dummy")
            # Scratch for VectorE's squared products. (tensor_mul +
            # reduce_sum, NOT tensor_tensor_reduce: TTR's accumulator-
            # read lowers to a raw InstISA op that crashes the HW path.)
            vout = small.tile([P, BIG], _DT, tag="vout")

            def alloc(ring_pool, ring_tag, w):
                if w == BIG:
                    return ring_pool.tile(
                        [P, w], _DT, tag=ring_tag, name=f"at_{ring_tag}"
                    )
                return tpools[w].tile(
                    [P, w], _DT, tag=f"{ring_tag}{w}", name=f"at_{ring_tag}{w}"
                )

            atA = []
            for t, c0, w in CHUNKS_A:
                at = alloc(bigA, "a", w)
                nc.sync.dma_start(out=at[:], in_=a_tiles[t][:, c0 : c0 + w])
                atA.append(at)

            atB = []

            def issue_b(k):
                t, c0, w = CHUNKS_B[k]
                at = alloc(bigB, "b", w)
                nc.gpsimd.dma_start(out=at[:], in_=a_tiles[t][:, c0 : c0 + w])
                atB.append(at)

            def act(tile_ap, col):
                nc.scalar.activation(
                    out=dummy.broadcast_to(tile_ap.shape),
                    in_=tile_ap[:],
                    func=mybir.ActivationFunctionType.Square,
                    accum_out=racc_a[:, col : col + 1],
                )

            def dve(k):
                # Single-pass square+reduce on VectorE: (at * 1.0) * at
                # with fused accumulator. DVE fp32 measures ~118 G
                # elem/s per pass, so the two-pass mul+reduce variant
                # made VectorE the critical path.
                at = atB[k]
                w = at.shape[1]
                nc.vector.scalar_tensor_tensor(
                    out=vout[:, :w],
                    in0=at[:],
                    scalar=1.0,
                    in1=at[:],
                    op0=mybir.AluOpType.mult,
                    op1=mybir.AluOpType.mult,
                    accum_out=racc_v[:, k : k + 1],
                )

            # Ring-B DMA triggers all live on the otherwise-idle GPSIMD
            # engine (SWDGE): buffer-reuse waits park there harmlessly,
            # and the ACT stream stays pure ACTIVATEs (in v7 the ring-B
            # triggers riding the ACT stream starved ring B).
            for k in range(len(CHUNKS_B)):
                issue_b(k)
            # ScalarE consumes ring A only; VectorE consumes ring B only.
            for k in range(NCHA):
                act(atA[k], k)
                dve(k)

            nc.sync.dma_start(out=out[:, :NCHA], in_=racc_a[:])
            nc.scalar.dma_start(out=out[:, NCHA:], in_=racc_v[:])

    nc.compile()
    return nc


_nc_cache = {}


def _get_nc():
    if "nc" not in _nc_cache:
        _nc_cache["nc"] = build_nc()
    return _nc_cache["nc"]


_COL_TILE = np.array([t for t, _, _ in CHUNKS_A] + [t for t, _, _ in CHUNKS_B])


def _run(inputs, trace=False):
    X = np.ascontiguousarray(np.asarray(inputs["X"], dtype=np.float32))
    A = np.ascontiguousarray(np.asarray(inputs["A"], dtype=np.float32))

    nc = _get_nc()
    in_maps = [
        {"a_shard": A[c * ROWS : (c + 1) * ROWS]} for c in range(NCORES)
    ]
    res = run_bass_kernel_spmd(
        nc, in_maps, core_ids=list(range(NCORES)), trace=trace
    )

    X64 = X.astype(np.float64)
    total = 0.0
    for c in range(NCORES):
        r = res.results[c]["out"].astype(np.float64)
        rt = np.zeros((P, TILES), dtype=np.float64)
        for col in range(NCH):
            rt[:, _COL_TILE[col]] += r[:, col]
        xc = X64[c * ROWS : (c + 1) * ROWS].reshape(TILES, P).T
        total += (xc * xc * rt).sum()

    d64 = np.asarray(A.diagonal(), dtype=np.float64)
    total += -2.0 * float(X64 @ d64) + float(N)
    return np.float32(total), res


def kernel(**inputs):
    out, _ = _run(inputs, trace=False)
    return out
